# revision 50
# baseline (speedup 1.0000x reference)
"""Causal self-attention (B=2, T=4096, C=768, H=12, D=64, RoPE) on 8 TRN2 cores.

Sharding: core c handles batch b = c//4 and heads [3g, 3g+1, 3g+2] with g = c%4
(data parallel over B, tensor parallel over heads). Each core computes its
heads' QKV projections, RoPE, causal attention and the partial output
projection; the host sums the 4 partial projections per batch.

Device-side layouts (per core):
  - x is shipped transposed and pre-cast to bf16: xT [768, 4096].
  - q/k are produced channel-major (qT [192, 4096]) with a per-head permuted
    channel order [even d | odd d] so RoPE becomes full-width elementwise ops
    plus a 32-partition swap done by SBUF->SBUF DMA. The 384 q+k channels are
    computed in 3 chunks of 128 rows (chunk 1 spans the q/k boundary).
  - v^T is produced directly on the PE (x-chunk stationary, Wv moving) into
    the ones-augmented v_aug layout (65 cols per head per key chunk) so row 64
    of the P^T@V accumulator is the softmax denominator.
  - attention computes S^T (keys on partitions, queries on free dim), exp on
    the scalar engine straight out of PSUM; each phase B chunk is followed by
    the attention superchunk it unblocks. Diagonal-chunk masking is a gpsimd
    memset of the fully-masked columns plus one 128x128 triangular multiply
    on DVE; the S/PV matmuls skip the fully-masked columns.
  - the output projection runs at the tail and emits outT [768, 4096]
    (partial over channels, bf16; the host sums the 4 partials per batch).

All matmuls run in bfloat16 with fp32 PSUM accumulation (~4e-3 rel err,
2x the fp32r column rate and half the weight-load and DMA cost).
"""

import sys

sys.path.insert(0, "/opt/trn_rl_repo")

from contextlib import ExitStack

import numpy as np
import ml_dtypes

import concourse.bass as bass
import concourse.tile as tile
from concourse import bacc, mybir
from concourse.bass_utils import run_bass_kernel_spmd

P = 128
C = 768
D = 64
HPC = 3            # heads per core
DQ = HPC * D       # 192 channels per core
W3 = 3 * DQ        # 576 = q+k+v output channels per core
KCH = C // P       # 6 contraction chunks for projections
TCH = 512          # phase B column chunk
SCQ = 512          # query superchunk (attention free dim)
GK = 2             # S^T tiles per exp group ([128, 1024] PSUM = 2 banks)
VW = HPC * 65      # v_aug row width per key chunk (3 heads x (64 + ones))

f32 = mybir.dt.float32
bf16 = mybir.dt.bfloat16
EXP = mybir.ActivationFunctionType.Exp
LN = mybir.ActivationFunctionType.Ln


def build(T=4096, n_cores=8):
    NT = T // TCH          # phase B chunks
    NSC = T // SCQ         # query superchunks
    nc = bacc.Bacc("TRN2", target_bir_lowering=False, debug=False,
                   num_devices=n_cores)

    xT_d = nc.dram_tensor("xT", [C, T], bf16, kind="ExternalInput").ap()
    w_d = nc.dram_tensor("w", [C, W3], bf16, kind="ExternalInput").ap()
    wp_d = nc.dram_tensor("wp", [DQ, C], bf16, kind="ExternalInput").ap()
    cp_d = nc.dram_tensor("cpat", [P, T], f32, kind="ExternalInput").ap()
    sp_d = nc.dram_tensor("spat", [P, T], f32, kind="ExternalInput").ap()
    mk_d = nc.dram_tensor("mkt", [P, P], bf16, kind="ExternalInput").ap()
    out_d = nc.dram_tensor("outT", [C, T], bf16, kind="ExternalOutput").ap()

    xT_v = xT_d.rearrange("(a p) t -> p a t", p=P)
    w_v = w_d.rearrange("(a p) w -> p a w", p=P)

    with tile.TileContext(nc) as tc, ExitStack() as top:
        const = top.enter_context(tc.tile_pool(name="const", bufs=1))
        persist = top.enter_context(tc.tile_pool(name="persist", bufs=1))

        # --- constants (loaded straight from bf16 HBM, no casts) ---
        w_r = const.tile([P, KCH, W3], bf16)
        wpA = const.tile([P, C], bf16)
        wpB = const.tile([DQ - P, C], bf16)
        mkt = const.tile([P, P], bf16)
        cpat = const.tile([P, T], f32)
        spat = const.tile([P, T], f32)
        # first q/k matmul only needs the first half of w and x chunk 0;
        # split those loads so the PE starts sooner
        nc.sync.dma_start(w_r[:, 0:3, :], w_v[:, 0:3, :])
        nc.sync.dma_start(w_r[:, 3:KCH, :], w_v[:, 3:KCH, :])
        # small late-phase constants go on the idle scalar queue so they are
        # not stuck behind the big x/cos/sin loads on the sync queue
        nc.scalar.dma_start(wpA[:], wp_d[0:P, :])
        nc.scalar.dma_start(wpB[:], wp_d[P:DQ, :])
        nc.scalar.dma_start(mkt[:], mk_d[:])

        # --- persistent activations ---
        xall = persist.tile([P, KCH, T], bf16)  # whole x^T, SBUF-resident
        # stage the big input loads in per-chunk lockstep so phase B chunk n
        # only waits for its own slices; wp/masks (needed later) come last
        for n in range(NT):
            cols = slice(n * TCH, (n + 1) * TCH)
            nc.sync.dma_start(xall[:, :, cols], xT_v[:, :, cols])
            nc.sync.dma_start(cpat[:, cols], cp_d[:, cols])
            nc.sync.dma_start(spat[:, cols], sp_d[:, cols])
        qt1 = persist.tile([P, T], bf16)      # q rows 0-127 (heads 0,1)
        kt1 = persist.tile([P, T], bf16)      # k rows 0-127 (heads 0,1)
        qt2 = persist.tile([D, T], bf16)      # q head 2
        kt2 = persist.tile([D, T], bf16)      # k head 2
        v_aug = persist.tile([P, (T // P) * VW], bf16)
        yt1 = persist.tile([P, T], bf16)      # y^T heads 0,1
        yt2 = persist.tile([D, T], bf16)      # y^T head 2

        ones_view = v_aug[:].rearrange(
            "p (a h c) -> p a h c", h=HPC, c=65)[:, :, :, 64]
        nc.gpsimd.memset(ones_view, 1.0)

        # q/k chunks of w columns: (offset, [(rows, dst_tile, dst_row0)...])
        def qk_chunks(n):
            c0 = slice(n * TCH, (n + 1) * TCH)
            return [
                (0, [(P, qt1, 0, c0)]),
                (P, [(D, qt2, 0, c0), (D, kt1, 0, c0)]),
                (2 * P, [(D, kt1, D, c0), (D, kt2, 0, c0)]),
            ]

        # ------- unified phase: per 512-col chunk, QKV+rope+v^T then the -------
        # ------- attention superchunk it unblocks; projection at the tail -----
        with ExitStack() as cctx:
            rtmp = cctx.enter_context(tc.tile_pool(name="rtmp", bufs=4))
            ps_ps = cctx.enter_context(
                tc.tile_pool(name="ps_ps", bufs=2, space="PSUM"))
            s_ps = cctx.enter_context(
                tc.tile_pool(name="s_ps", bufs=2, space="PSUM"))
            y_ps = cctx.enter_context(
                tc.tile_pool(name="y_ps", bufs=2, space="PSUM"))
            ptp = cctx.enter_context(tc.tile_pool(name="ptp", bufs=3))
            smp = cctx.enter_context(tc.tile_pool(name="smp", bufs=2))
            otp = cctx.enter_context(tc.tile_pool(name="otp", bufs=3))

            def phase_b_chunk(n):
                cols = slice(n * TCH, (n + 1) * TCH)
                xr = xall[:, :, cols]
                cpc = cpat[:, cols]
                spc = spat[:, cols]

                # q/k: W stationary (channel-major out), rope applied in PSUM.
                # Matmuls + muls + swap DMAs for all 3 chunks are emitted
                # before any add, so the swap-DMA round trip hides behind the
                # other chunks' DVE work (in-order engine queues).
                adds = []
                for moff, dsts in qk_chunks(n):
                    ps = ps_ps.tile([P, TCH], f32, tag="ps")
                    for kc in range(KCH):
                        nc.tensor.matmul(ps[:], w_r[:, kc, moff:moff + P],
                                         xr[:, kc, :],
                                         start=(kc == 0), stop=(kc == KCH - 1))
                    # rope: out = psum*cpat + swap32(psum*spat)
                    ct = rtmp.tile([P, TCH], bf16, tag="ct")
                    st = rtmp.tile([P, TCH], bf16, tag="st")
                    wt = rtmp.tile([P, TCH], bf16, tag="wt")
                    nc.vector.tensor_mul(ct[:], ps[:], cpc)
                    nc.vector.tensor_mul(st[:], ps[:], spc)
                    for blk in range(P // D):
                        p0 = blk * D
                        nc.gpsimd.dma_start(wt[p0:p0 + 32, :],
                                            st[p0 + 32:p0 + D, :])
                        nc.gpsimd.dma_start(wt[p0 + 32:p0 + D, :],
                                            st[p0:p0 + 32, :])
                    adds.append((ct, wt, dsts))

                # v^T: x chunk stationary, Wv moving -> [128 t, 192 ch]
                for tt in range(TCH // P):
                    kc32 = n * (TCH // P) + tt
                    pv = ps_ps.tile([P, TCH], f32, tag="ps")
                    for kc in range(KCH):
                        nc.tensor.matmul(
                            pv[:, 0:DQ], xr[:, kc, tt * P:(tt + 1) * P],
                            w_r[:, kc, 2 * DQ:W3],
                            start=(kc == 0), stop=(kc == KCH - 1))
                    vdst = v_aug[:, kc32 * VW:(kc32 + 1) * VW].rearrange(
                        "p (h c) -> p h c", h=HPC)[:, :, 0:D]
                    nc.scalar.copy(
                        vdst, pv[:, 0:DQ].rearrange("p (h c) -> p h c", h=HPC))

                for ct, wt, dsts in adds:
                    r0 = 0
                    for rows, dtile, drow, dcols in dsts:
                        nc.vector.tensor_add(dtile[drow:drow + rows, dcols],
                                             ct[r0:r0 + rows, :],
                                             wt[r0:r0 + rows, :])
                        r0 += rows

            def attention(h, s):
                if h < 2:
                    q_rows = qt1[h * D:(h + 1) * D, :]
                    k_rows = kt1[h * D:(h + 1) * D, :]
                else:
                    q_rows = qt2[:, :]
                    k_rows = kt2[:, :]
                scols = slice(s * SCQ, (s + 1) * SCQ)
                q_ap = q_rows[:, scols]
                psy = y_ps.tile([65, SCQ], f32, tag="y")
                nkj = 4 * s + 4
                pending = None  # (pt tile, g0, gsz) awaiting PV
                for g0 in range(0, nkj, GK):
                    gsz = min(GK, nkj - g0)
                    pss = s_ps.tile([P, GK * SCQ], f32, tag="ss")
                    for j in range(gsz):
                        kj = g0 + j
                        # queries below column 128*dg of a diagonal chunk are
                        # fully masked (the memset below zeroes them), so the
                        # S matmul skips those columns
                        q0 = P * (kj - 4 * s) if kj >= 4 * s else 0
                        nc.tensor.matmul(
                            pss[:, j * SCQ + q0:(j + 1) * SCQ],
                            k_rows[:, kj * P:(kj + 1) * P], q_ap[:, q0:],
                            start=True, stop=True)
                    pt = ptp.tile([P, GK * SCQ], bf16, tag="pt")
                    nc.scalar.activation(pt[:, :gsz * SCQ],
                                         pss[:, :gsz * SCQ], EXP,
                                         scale=0.125)
                    for j in range(gsz):
                        kj = g0 + j
                        if kj >= 4 * s:
                            # diagonal chunk: queries before this key block
                            # are fully masked (gpsimd memset, off the DVE
                            # queue); the 128x128 block on the diagonal gets
                            # the triangular mask on DVE
                            dg = kj - 4 * s
                            j0 = j * SCQ
                            if dg > 0:
                                nc.gpsimd.memset(pt[:, j0:j0 + P * dg], 0.0)
                            dc = slice(j0 + P * dg, j0 + P * (dg + 1))
                            nc.vector.tensor_mul(pt[:, dc], pt[:, dc],
                                                 mkt[:])
                    if pending is not None:
                        _emit_pv(nc, psy, v_aug, pending, h, nkj)
                    pending = (pt, g0, gsz)
                _emit_pv(nc, psy, v_aug, pending, h, nkj)

                rl = smp.tile([1, SCQ], f32, tag="rl")
                nc.vector.reciprocal(rl[:], psy[64:65, :])
                rlb = smp.tile([D, SCQ], f32, tag="rlb")
                nc.gpsimd.partition_broadcast(rlb[:], rl[:])
                ydst = (yt1[h * D:(h + 1) * D, scols] if h < 2
                        else yt2[:, scols])
                nc.vector.tensor_mul(ydst, psy[0:D, :], rlb[:])

            def emit_proj(s):
                # projection for column block s: outT = wp.T @ y^T
                scols = slice(s * SCQ, (s + 1) * SCQ)
                for m in range(C // P):
                    pso = ps_ps.tile([P, TCH], f32, tag="ps")
                    nc.tensor.matmul(pso[:, 0:SCQ], wpA[:, m * P:(m + 1) * P],
                                     yt1[:, scols], start=True, stop=False)
                    nc.tensor.matmul(pso[:, 0:SCQ], wpB[:, m * P:(m + 1) * P],
                                     yt2[:, scols], start=False, stop=True)
                    ot = otp.tile([P, SCQ], bf16, tag="ot")
                    nc.vector.tensor_copy(ot[:], pso[:, 0:SCQ])
                    nc.sync.dma_start(out_d[m * P:(m + 1) * P, scols], ot[:])

            # round n: phase B chunk n (rope DVE work first so the ps PSUM
            # ring recycles promptly), then the attention superchunk it
            # unblocks, then projection two rounds behind (no pending deps,
            # so it fills PE stalls while early attention waits on DVE).
            # projection at the tail: interleaving it couples the shared
            # "ps" PSUM ring to the q/k matmuls and stalls the next round
            for n in range(NT):
                phase_b_chunk(n)
                for h in range(HPC):
                    attention(h, n)
            for s in range(NSC):
                emit_proj(s)

    nc.compile()
    return nc


def _emit_pv(nc, psy, v_aug, pending, h, nkj):
    pt, g0, gsz = pending
    s4 = nkj - 4
    for j in range(gsz):
        kj = g0 + j
        # diagonal chunks contribute zeros to the first 128*dg columns
        # (those pt entries are memset to 0), so skip them — except for
        # kj == 0 which must reset the whole accumulator
        q0 = P * (kj - s4) if (kj > s4 and kj > 0) else 0
        nc.tensor.matmul(psy[:, q0:],
                         v_aug[:, kj * VW + h * 65: kj * VW + (h + 1) * 65],
                         pt[:, j * SCQ + q0:(j + 1) * SCQ],
                         start=(kj == 0), stop=(kj == nkj - 1))


# ---------------------------------------------------------------------------
# host side
# ---------------------------------------------------------------------------

def make_core_inputs(x, Wq, bq, Wk, bk, Wv, bv, Wp, bp, T=4096, n_cores=8):
    """Build the per-core input maps. Biases bq/bk/bv must be zero (they are
    for this problem); bv/bp are folded on the host in kernel()."""
    H = 12
    b16 = ml_dtypes.bfloat16
    cpat = np.empty((P, T), dtype=np.float32)
    spat = np.empty((P, T), dtype=np.float32)
    inv_freq = (10000.0 ** (-(np.arange(32, dtype=np.float64)) / 32.0))
    ang = np.arange(T, dtype=np.float64)[None, :] * inv_freq[:, None]  # [32,T]
    cos32 = np.cos(ang).astype(np.float32)
    sin32 = np.sin(ang).astype(np.float32)
    for blk in range(4):
        cpat[blk * 32:(blk + 1) * 32] = cos32
        spat[blk * 32:(blk + 1) * 32] = sin32 if blk % 2 == 0 else -sin32

    jj = np.arange(P)[:, None]
    ii = np.arange(P)[None, :]
    mkt = (jj <= ii).astype(b16)

    in_maps = []
    for c in range(n_cores):
        b, g = divmod(c, n_cores // 2)
        heads = [HPC * g + i for i in range(HPC)]
        qk_rows = []
        v_rows = []
        for h in heads:
            base = D * h
            qk_rows += [base + 2 * i for i in range(32)]
            qk_rows += [base + 2 * i + 1 for i in range(32)]
            v_rows += list(range(base, base + D))
        w_cat = np.concatenate(
            [Wq[qk_rows, :].T, Wk[qk_rows, :].T, Wv[v_rows, :].T],
            axis=1).astype(b16)
        wp_s = np.ascontiguousarray(Wp[:, v_rows].T.astype(b16))
        xT = np.ascontiguousarray(x[b].T.astype(b16))
        im = {
            "xT": xT, "w": np.ascontiguousarray(w_cat), "wp": wp_s,
            "cpat": cpat, "spat": spat, "mkt": mkt,
        }
        in_maps.append(im)
    return in_maps


_nc_cache = {}


def run(x, Wq, bq, Wk, bk, Wv, bv, Wp, bp, T=4096, n_cores=8, trace=False,
        trace_cores=None):
    assert not (np.any(bq) or np.any(bk)), "nonzero q/k bias unsupported"
    key = (T, n_cores)
    if key not in _nc_cache:
        _nc_cache[key] = build(T=T, n_cores=n_cores)
    nc = _nc_cache[key]
    in_maps = make_core_inputs(x, Wq, bq, Wk, bk, Wv, bv, Wp, bp,
                               T=T, n_cores=n_cores)
    res = run_bass_kernel_spmd(nc, in_maps, list(range(n_cores)), trace=trace,
                               trace_cores=trace_cores)
    B = 2
    out = np.zeros((B, T, C), dtype=np.float32)
    for c in range(n_cores):
        b = c // (n_cores // 2)
        out[b] += res.results[c]["outT"].T.astype(np.float32)
    # host-folded bias terms: softmax rows sum to 1, so the v bias passes
    # through attention unchanged: y = att@v + bv  =>  out += bv @ Wp.T + bp
    out += (bv.astype(np.float32) @ Wp.T.astype(np.float32) + bp)[None, None, :]
    return out, res


def kernel(**inputs):
    inputs = {k: np.asarray(v) for k, v in inputs.items()}
    out, _ = run(**inputs)
    return out


# revision 52
# speedup vs baseline: 1.0184x; 1.0184x over previous
"""Causal self-attention (B=2, T=4096, C=768, H=12, D=64, RoPE) on 8 TRN2 cores.

Sharding: core c handles batch b = c//4 and heads [3g, 3g+1, 3g+2] with g = c%4
(data parallel over B, tensor parallel over heads). Each core computes its
heads' QKV projections, RoPE, causal attention and the partial output
projection; the host sums the 4 partial projections per batch.

Device-side layouts (per core):
  - x is shipped transposed and pre-cast to bf16: xT [768, 4096].
  - q/k are produced channel-major (qT [192, 4096]) with a per-head permuted
    channel order [even d | odd d] so RoPE becomes full-width elementwise ops
    plus a 32-partition swap done by SBUF->SBUF DMA. The 384 q+k channels are
    computed in 3 chunks of 128 rows (chunk 1 spans the q/k boundary).
  - v^T is produced directly on the PE (x-chunk stationary, Wv moving) into
    the ones-augmented v_aug layout (65 cols per head per key chunk) so row 64
    of the P^T@V accumulator is the softmax denominator.
  - attention computes S^T (keys on partitions, queries on free dim), exp on
    the scalar engine straight out of PSUM; each phase B chunk is followed by
    the attention superchunk it unblocks. Diagonal-chunk masking is a gpsimd
    memset of the fully-masked columns plus one 128x128 triangular multiply
    on DVE; the S/PV matmuls skip the fully-masked columns.
  - the output projection runs at the tail and emits outT [768, 4096]
    (partial over channels, bf16; the host sums the 4 partials per batch).

All matmuls run in bfloat16 with fp32 PSUM accumulation (~4e-3 rel err,
2x the fp32r column rate and half the weight-load and DMA cost).
"""

import sys

sys.path.insert(0, "/opt/trn_rl_repo")

from contextlib import ExitStack

import numpy as np
import ml_dtypes

import concourse.bass as bass
import concourse.tile as tile
from concourse import bacc, mybir
from concourse.bass_utils import run_bass_kernel_spmd

P = 128
C = 768
D = 64
HPC = 3            # heads per core
DQ = HPC * D       # 192 channels per core
W3 = 3 * DQ        # 576 = q+k+v output channels per core
KCH = C // P       # 6 contraction chunks for projections
TCH = 512          # phase B column chunk
SCQ = 512          # query superchunk (attention free dim)
GK = 2             # S^T tiles per exp group ([128, 1024] PSUM = 2 banks)
VW = HPC * 65      # v_aug row width per key chunk (3 heads x (64 + ones))

f32 = mybir.dt.float32
bf16 = mybir.dt.bfloat16
EXP = mybir.ActivationFunctionType.Exp
LN = mybir.ActivationFunctionType.Ln


def build(T=4096, n_cores=8):
    NT = T // TCH          # phase B chunks
    NSC = T // SCQ         # query superchunks
    nc = bacc.Bacc("TRN2", target_bir_lowering=False, debug=False,
                   num_devices=n_cores)

    xT_d = nc.dram_tensor("xT", [C, T], bf16, kind="ExternalInput").ap()
    w_d = nc.dram_tensor("w", [C, W3], bf16, kind="ExternalInput").ap()
    wp_d = nc.dram_tensor("wp", [DQ, C], bf16, kind="ExternalInput").ap()
    cp_d = nc.dram_tensor("cpat", [P, T], f32, kind="ExternalInput").ap()
    sp_d = nc.dram_tensor("spat", [P, T], f32, kind="ExternalInput").ap()
    mk_d = nc.dram_tensor("mkt", [P, P], bf16, kind="ExternalInput").ap()
    out_d = nc.dram_tensor("outT", [C, T], bf16, kind="ExternalOutput").ap()

    xT_v = xT_d.rearrange("(a p) t -> p a t", p=P)
    w_v = w_d.rearrange("(a p) w -> p a w", p=P)

    with tile.TileContext(nc) as tc, ExitStack() as top:
        const = top.enter_context(tc.tile_pool(name="const", bufs=1))
        persist = top.enter_context(tc.tile_pool(name="persist", bufs=1))

        # --- constants (loaded straight from bf16 HBM, no casts) ---
        w_r = const.tile([P, KCH, W3], bf16)
        wpA = const.tile([P, C], bf16)
        wpB = const.tile([DQ - P, C], bf16)
        mkt = const.tile([P, P], bf16)
        cpat = const.tile([P, T], f32)
        spat = const.tile([P, T], f32)
        # first q/k matmul only needs the first half of w and x chunk 0;
        # split those loads so the PE starts sooner
        nc.sync.dma_start(w_r[:, 0:3, :], w_v[:, 0:3, :])
        nc.sync.dma_start(w_r[:, 3:KCH, :], w_v[:, 3:KCH, :])
        # small late-phase constants go on the idle scalar queue so they are
        # not stuck behind the big x/cos/sin loads on the sync queue
        nc.scalar.dma_start(wpA[:], wp_d[0:P, :])
        nc.scalar.dma_start(wpB[:], wp_d[P:DQ, :])
        nc.scalar.dma_start(mkt[:], mk_d[:])

        # --- persistent activations ---
        xall = persist.tile([P, KCH, T], bf16)  # whole x^T, SBUF-resident
        # stage the big input loads in per-chunk lockstep so phase B chunk n
        # only waits for its own slices; wp/masks (needed later) come last
        for n in range(NT):
            cols = slice(n * TCH, (n + 1) * TCH)
            nc.sync.dma_start(xall[:, :, cols], xT_v[:, :, cols])
            nc.sync.dma_start(cpat[:, cols], cp_d[:, cols])
            nc.sync.dma_start(spat[:, cols], sp_d[:, cols])
        qt1 = persist.tile([P, T], bf16)      # q rows 0-127 (heads 0,1)
        kt1 = persist.tile([P, T], bf16)      # k rows 0-127 (heads 0,1)
        qt2 = persist.tile([D, T], bf16)      # q head 2
        kt2 = persist.tile([D, T], bf16)      # k head 2
        v_aug = persist.tile([P, (T // P) * VW], bf16)
        yt1 = persist.tile([P, T], bf16)      # y^T heads 0,1
        yt2 = persist.tile([D, T], bf16)      # y^T head 2

        ones_view = v_aug[:].rearrange(
            "p (a h c) -> p a h c", h=HPC, c=65)[:, :, :, 64]
        nc.gpsimd.memset(ones_view, 1.0)

        # q/k chunks of w columns: (offset, [(rows, dst_tile, dst_row0)...])
        def qk_chunks(n):
            c0 = slice(n * TCH, (n + 1) * TCH)
            return [
                (0, [(P, qt1, 0, c0)]),
                (P, [(D, qt2, 0, c0), (D, kt1, 0, c0)]),
                (2 * P, [(D, kt1, D, c0), (D, kt2, 0, c0)]),
            ]

        # ------- unified phase: per 512-col chunk, QKV+rope+v^T then the -------
        # ------- attention superchunk it unblocks; projection at the tail -----
        with ExitStack() as cctx:
            rtmp = cctx.enter_context(tc.tile_pool(name="rtmp", bufs=4))
            ps_ps = cctx.enter_context(
                tc.tile_pool(name="ps_ps", bufs=2, space="PSUM"))
            s_ps = cctx.enter_context(
                tc.tile_pool(name="s_ps", bufs=2, space="PSUM"))
            y_ps = cctx.enter_context(
                tc.tile_pool(name="y_ps", bufs=2, space="PSUM"))
            ptp = cctx.enter_context(tc.tile_pool(name="ptp", bufs=3))
            smp = cctx.enter_context(tc.tile_pool(name="smp", bufs=2))
            otp = cctx.enter_context(tc.tile_pool(name="otp", bufs=3))

            def phase_b_chunk(n):
                cols = slice(n * TCH, (n + 1) * TCH)
                xr = xall[:, :, cols]
                cpc = cpat[:, cols]
                spc = spat[:, cols]

                # q/k: W stationary (channel-major out), rope applied in PSUM.
                # Matmuls + muls + swap DMAs for all 3 chunks are emitted
                # before any add, so the swap-DMA round trip hides behind the
                # other chunks' DVE work (in-order engine queues).
                adds = []
                for moff, dsts in qk_chunks(n):
                    ps = ps_ps.tile([P, TCH], f32, tag="ps")
                    for kc in range(KCH):
                        nc.tensor.matmul(ps[:], w_r[:, kc, moff:moff + P],
                                         xr[:, kc, :],
                                         start=(kc == 0), stop=(kc == KCH - 1))
                    # rope: out = psum*cpat + swap32(psum*spat)
                    ct = rtmp.tile([P, TCH], bf16, tag="ct")
                    st = rtmp.tile([P, TCH], bf16, tag="st")
                    wt = rtmp.tile([P, TCH], bf16, tag="wt")
                    nc.vector.tensor_mul(ct[:], ps[:], cpc)
                    nc.vector.tensor_mul(st[:], ps[:], spc)
                    for blk in range(P // D):
                        p0 = blk * D
                        nc.gpsimd.dma_start(wt[p0:p0 + 32, :],
                                            st[p0 + 32:p0 + D, :])
                        nc.gpsimd.dma_start(wt[p0 + 32:p0 + D, :],
                                            st[p0:p0 + 32, :])
                    adds.append((ct, wt, dsts))

                # v^T: x chunk stationary, Wv moving -> [128 t, 192 ch]
                for tt in range(TCH // P):
                    kc32 = n * (TCH // P) + tt
                    pv = ps_ps.tile([P, TCH], f32, tag="ps")
                    for kc in range(KCH):
                        nc.tensor.matmul(
                            pv[:, 0:DQ], xr[:, kc, tt * P:(tt + 1) * P],
                            w_r[:, kc, 2 * DQ:W3],
                            start=(kc == 0), stop=(kc == KCH - 1))
                    vdst = v_aug[:, kc32 * VW:(kc32 + 1) * VW].rearrange(
                        "p (h c) -> p h c", h=HPC)[:, :, 0:D]
                    nc.scalar.copy(
                        vdst, pv[:, 0:DQ].rearrange("p (h c) -> p h c", h=HPC))

                for ct, wt, dsts in adds:
                    r0 = 0
                    for rows, dtile, drow, dcols in dsts:
                        nc.vector.tensor_add(dtile[drow:drow + rows, dcols],
                                             ct[r0:r0 + rows, :],
                                             wt[r0:r0 + rows, :])
                        r0 += rows

            def attention(h, s):
                if h < 2:
                    q_rows = qt1[h * D:(h + 1) * D, :]
                    k_rows = kt1[h * D:(h + 1) * D, :]
                else:
                    q_rows = qt2[:, :]
                    k_rows = kt2[:, :]
                scols = slice(s * SCQ, (s + 1) * SCQ)
                q_ap = q_rows[:, scols]
                psy = y_ps.tile([65, SCQ], f32, tag="y")
                nkj = 4 * s + 4
                pending = None  # (pt tile, g0, gsz) awaiting PV
                for g0 in range(0, nkj, GK):
                    gsz = min(GK, nkj - g0)
                    pss = s_ps.tile([P, GK * SCQ], f32, tag="ss")
                    for j in range(gsz):
                        kj = g0 + j
                        # queries below column 128*dg of a diagonal chunk are
                        # fully masked (the memset below zeroes them), so the
                        # S matmul skips those columns
                        q0 = P * (kj - 4 * s) if kj >= 4 * s else 0
                        nc.tensor.matmul(
                            pss[:, j * SCQ + q0:(j + 1) * SCQ],
                            k_rows[:, kj * P:(kj + 1) * P], q_ap[:, q0:],
                            start=True, stop=True)
                    pt = ptp.tile([P, GK * SCQ], bf16, tag="pt")
                    nc.scalar.activation(pt[:, :gsz * SCQ],
                                         pss[:, :gsz * SCQ], EXP,
                                         scale=0.125)
                    for j in range(gsz):
                        kj = g0 + j
                        if kj >= 4 * s:
                            # diagonal chunk: queries before this key block
                            # are fully masked (gpsimd memset, off the DVE
                            # queue); the 128x128 block on the diagonal gets
                            # the triangular mask on DVE
                            dg = kj - 4 * s
                            j0 = j * SCQ
                            if dg > 0:
                                nc.gpsimd.memset(pt[:, j0:j0 + P * dg], 0.0)
                            dc = slice(j0 + P * dg, j0 + P * (dg + 1))
                            nc.vector.tensor_mul(pt[:, dc], pt[:, dc],
                                                 mkt[:])
                    if pending is not None:
                        _emit_pv(nc, psy, v_aug, pending, h, nkj)
                    pending = (pt, g0, gsz)
                _emit_pv(nc, psy, v_aug, pending, h, nkj)

                def norm_fn():
                    # reciprocal + normalize, emitted one attention step
                    # later by the driver so the 3.3us DVE reciprocal never
                    # sits ahead of the next superchunk's mask multiplies in
                    # the in-order vector queue (y is only consumed by the
                    # tail projection)
                    rl = smp.tile([1, SCQ], f32, tag="rl")
                    nc.vector.reciprocal(rl[:], psy[64:65, :])
                    rlb = smp.tile([D, SCQ], f32, tag="rlb")
                    nc.gpsimd.partition_broadcast(rlb[:], rl[:])
                    ydst = (yt1[h * D:(h + 1) * D, scols] if h < 2
                            else yt2[:, scols])
                    nc.vector.tensor_mul(ydst, psy[0:D, :], rlb[:])
                return norm_fn

            def emit_proj(s):
                # projection for column block s: outT = wp.T @ y^T
                scols = slice(s * SCQ, (s + 1) * SCQ)
                for m in range(C // P):
                    pso = ps_ps.tile([P, TCH], f32, tag="ps")
                    nc.tensor.matmul(pso[:, 0:SCQ], wpA[:, m * P:(m + 1) * P],
                                     yt1[:, scols], start=True, stop=False)
                    nc.tensor.matmul(pso[:, 0:SCQ], wpB[:, m * P:(m + 1) * P],
                                     yt2[:, scols], start=False, stop=True)
                    ot = otp.tile([P, SCQ], bf16, tag="ot")
                    nc.vector.tensor_copy(ot[:], pso[:, 0:SCQ])
                    nc.sync.dma_start(out_d[m * P:(m + 1) * P, scols], ot[:])

            # round n: phase B chunk n (rope DVE work first so the ps PSUM
            # ring recycles promptly), then the attention superchunk it
            # unblocks, then projection two rounds behind (no pending deps,
            # so it fills PE stalls while early attention waits on DVE).
            # projection at the tail: interleaving it couples the shared
            # "ps" PSUM ring to the q/k matmuls and stalls the next round
            pending_norm = None
            for n in range(NT):
                phase_b_chunk(n)
                for h in range(HPC):
                    nf = attention(h, n)
                    if pending_norm is not None:
                        pending_norm()
                    pending_norm = nf
            pending_norm()
            for s in range(NSC):
                emit_proj(s)

    nc.compile()
    return nc


def _emit_pv(nc, psy, v_aug, pending, h, nkj):
    pt, g0, gsz = pending
    s4 = nkj - 4
    for j in range(gsz):
        kj = g0 + j
        # diagonal chunks contribute zeros to the first 128*dg columns
        # (those pt entries are memset to 0), so skip them — except for
        # kj == 0 which must reset the whole accumulator
        q0 = P * (kj - s4) if (kj > s4 and kj > 0) else 0
        nc.tensor.matmul(psy[:, q0:],
                         v_aug[:, kj * VW + h * 65: kj * VW + (h + 1) * 65],
                         pt[:, j * SCQ + q0:(j + 1) * SCQ],
                         start=(kj == 0), stop=(kj == nkj - 1))


# ---------------------------------------------------------------------------
# host side
# ---------------------------------------------------------------------------

def make_core_inputs(x, Wq, bq, Wk, bk, Wv, bv, Wp, bp, T=4096, n_cores=8):
    """Build the per-core input maps. Biases bq/bk/bv must be zero (they are
    for this problem); bv/bp are folded on the host in kernel()."""
    H = 12
    b16 = ml_dtypes.bfloat16
    cpat = np.empty((P, T), dtype=np.float32)
    spat = np.empty((P, T), dtype=np.float32)
    inv_freq = (10000.0 ** (-(np.arange(32, dtype=np.float64)) / 32.0))
    ang = np.arange(T, dtype=np.float64)[None, :] * inv_freq[:, None]  # [32,T]
    cos32 = np.cos(ang).astype(np.float32)
    sin32 = np.sin(ang).astype(np.float32)
    for blk in range(4):
        cpat[blk * 32:(blk + 1) * 32] = cos32
        spat[blk * 32:(blk + 1) * 32] = sin32 if blk % 2 == 0 else -sin32

    jj = np.arange(P)[:, None]
    ii = np.arange(P)[None, :]
    mkt = (jj <= ii).astype(b16)

    in_maps = []
    for c in range(n_cores):
        b, g = divmod(c, n_cores // 2)
        heads = [HPC * g + i for i in range(HPC)]
        qk_rows = []
        v_rows = []
        for h in heads:
            base = D * h
            qk_rows += [base + 2 * i for i in range(32)]
            qk_rows += [base + 2 * i + 1 for i in range(32)]
            v_rows += list(range(base, base + D))
        w_cat = np.concatenate(
            [Wq[qk_rows, :].T, Wk[qk_rows, :].T, Wv[v_rows, :].T],
            axis=1).astype(b16)
        wp_s = np.ascontiguousarray(Wp[:, v_rows].T.astype(b16))
        xT = np.ascontiguousarray(x[b].T.astype(b16))
        im = {
            "xT": xT, "w": np.ascontiguousarray(w_cat), "wp": wp_s,
            "cpat": cpat, "spat": spat, "mkt": mkt,
        }
        in_maps.append(im)
    return in_maps


_nc_cache = {}


def run(x, Wq, bq, Wk, bk, Wv, bv, Wp, bp, T=4096, n_cores=8, trace=False,
        trace_cores=None):
    assert not (np.any(bq) or np.any(bk)), "nonzero q/k bias unsupported"
    key = (T, n_cores)
    if key not in _nc_cache:
        _nc_cache[key] = build(T=T, n_cores=n_cores)
    nc = _nc_cache[key]
    in_maps = make_core_inputs(x, Wq, bq, Wk, bk, Wv, bv, Wp, bp,
                               T=T, n_cores=n_cores)
    res = run_bass_kernel_spmd(nc, in_maps, list(range(n_cores)), trace=trace,
                               trace_cores=trace_cores)
    B = 2
    out = np.zeros((B, T, C), dtype=np.float32)
    for c in range(n_cores):
        b = c // (n_cores // 2)
        out[b] += res.results[c]["outT"].T.astype(np.float32)
    # host-folded bias terms: softmax rows sum to 1, so the v bias passes
    # through attention unchanged: y = att@v + bv  =>  out += bv @ Wp.T + bp
    out += (bv.astype(np.float32) @ Wp.T.astype(np.float32) + bp)[None, None, :]
    return out, res


def kernel(**inputs):
    inputs = {k: np.asarray(v) for k, v in inputs.items()}
    out, _ = run(**inputs)
    return out
